# revision 9
# baseline (speedup 1.0000x reference)
"""Trainium2 Bass kernel for grouped-correlation cost volume (GwcNet style).

cost[b,g,d,h,w] = mean_{c in group g}( ref[b,c,h,w] * tgt[b,c,h,w-d] ), 0 if w<d

Hardcoded problem size: B=4, C=320, H=64, W=128, D=48, G=40 (cg=8), f32.
Sharding: 8 cores = (4 batches) x (2 halves of H). Each core computes its
[G, D, 32, W] shard; host reassembles.

Per-core dataflow (engines balanced, DVE does ONLY the multiplies):
  - inputs are cast f32->bf16 during the load DMA (SWDGE) into a 5-chunk
    packing that keeps all 128 partitions busy: chunks = (ch0:128 r0:16),
    (ch128:256 r0:16), (ch256:320 r0:16 | ch0:64 r16:32), (ch64:192 r16:32),
    (ch192:320 r16:32). tgt is loaded twice into zero-padded tiles at even
    and odd column parity so any disparity shift is a 4B-aligned slice
    (DVE 2x mode).
  - per d: ONE DVE tensor_mul over the whole 5-chunk stack [128, 5, 16, 128].
  - group-sum on PE: block-diagonal (1/8) ones matmuls; 3 consecutive d are
    packed into one PSUM tile [120, 4, 128] (partition = 40*d_j + g).
  - ScalarE (ACT) evacuates PSUM->SBUF; HWDGE DMAs stream SBUF->DRAM out.
"""

import os
import sys

if "/opt/trn_rl_repo" not in sys.path:
    sys.path.insert(0, "/opt/trn_rl_repo")

import numpy as np

B, C, H, W = 4, 320, 64, 128
D, G, CG = 48, 40, 8
NCORES = 8
Hc = H // 2      # 32 rows of h per core
NCH = 5          # channel-row chunks, each 128 partitions
HB = 16          # h rows per chunk
PADW_E = 176     # even-parity padded tgt width (data at cols 48..176)
PADW_O = 178     # odd-parity padded tgt width (data at cols 49..177)
NT = D // 3      # d-triples
NR = Hc // 4     # 4-row output regions per d

# (chunk k, part0, parts, ch0, row0) for the input loads
_LOADS = [
    (0, 0, 128, 0, 0),
    (1, 0, 128, 128, 0),
    (2, 0, 64, 256, 0),
    (2, 64, 64, 0, 16),
    (3, 0, 128, 64, 16),
    (4, 0, 128, 192, 16),
]

_CACHE = {}
LAST_RESULT = None  # BassKernelResults of the most recent run (for profiling)


def _make_ones():
    import ml_dtypes

    # ones[p, col, j, 40*j+g]: block-diagonal 1/8 for each chunk's
    # channel->group map, replicated at d-block offset j so matmul dests can
    # stay at PSUM base partition 0 (HW allows base 0/32/64 only).
    ones = np.zeros((128, 6, 3, 120), dtype=ml_dtypes.bfloat16)
    for p in range(128):
        for j in range(3):
            o = 40 * j
            ones[p, 0, j, o + p // CG] = 0.125             # chunk 0: ch p
            ones[p, 1, j, o + 16 + p // CG] = 0.125        # chunk 1: ch 128+p
            if p < 64:
                ones[p, 2, j, o + 32 + p // CG] = 0.125    # chunk 2 lo: ch 256+p
            else:
                ones[p, 3, j, o + (p - 64) // CG] = 0.125  # chunk 2 hi: ch p-64
            ones[p, 4, j, o + 8 + p // CG] = 0.125         # chunk 3: ch 64+p
            ones[p, 5, j, o + 24 + p // CG] = 0.125        # chunk 4: ch 192+p
    return ones


def _build_nc():
    import concourse.bass as bass
    import concourse.mybir as mybir
    from concourse import tile

    nc = bass.Bass()
    ref_d = nc.dram_tensor("ref", [C, Hc, W], mybir.dt.float32, kind="ExternalInput")
    tgt_d = nc.dram_tensor("tgt", [C, Hc, W], mybir.dt.float32, kind="ExternalInput")
    ones_d = nc.dram_tensor("ones", [128, 6, 3, 120], mybir.dt.bfloat16, kind="ExternalInput")
    out_d = nc.dram_tensor("out", [G, D, Hc, W], mybir.dt.float32, kind="ExternalOutput")

    bf16 = mybir.dt.bfloat16
    f32 = mybir.dt.float32

    with tile.TileContext(nc) as tc:
        with (
            tc.tile_pool(name="const", bufs=1) as constp,
            tc.tile_pool(name="inp", bufs=1) as inp,
            tc.tile_pool(name="prodp", bufs=4) as prodp,
            tc.tile_pool(name="outp", bufs=8) as outp,
            tc.tile_pool(name="psum", bufs=8, space="PSUM") as psump,
        ):
            ones_sb = constp.tile([128, 6, 3, 120], bf16)
            nc.sync.dma_start(ones_sb[:], ones_d[:])
            # zero stationary/moving operands for the WAR-carrier dummy
            # matmuls (see below); K=1 so they cost ~N cycles each.
            zer_lhs = constp.tile([1, 120], bf16)
            zer_rhs = constp.tile([1, 4, W], bf16)
            nc.gpsimd.memset(zer_lhs[:], 0.0)
            nc.gpsimd.memset(zer_rhs[:], 0.0)

            rf = inp.tile([128, NCH, HB, W], bf16)
            tge = inp.tile([128, NCH, HB, PADW_E], bf16)
            tgo = inp.tile([128, NCH, HB, PADW_O], bf16)

            # zero the padded tgt tiles (pads must be 0), then cast-load the
            # data columns over them. gpsimd so DVE stays free.
            nc.gpsimd.memset(tge[:], 0.0)
            nc.gpsimd.memset(tgo[:], 0.0)
            for k, p0, np_, c0, r0 in _LOADS:
                src = lambda t: t[c0 : c0 + np_, r0 : r0 + HB, :]
                nc.gpsimd.dma_start(rf[p0 : p0 + np_, k, :, :], src(ref_d))
                nc.gpsimd.dma_start(tge[p0 : p0 + np_, k, :, 48 : 48 + W], src(tgt_d))
                nc.gpsimd.dma_start(tgo[p0 : p0 + np_, k, :, 49 : 49 + W], src(tgt_d))

            # tiny DVE joins: one per input writer, so no later (big) DVE
            # instruction ever needs more than a couple of sync waits.
            for k, p0, np_, c0, r0 in _LOADS:
                nc.vector.tensor_copy(rf[p0 : p0 + 1, k, 0:1, 0:2], rf[p0 : p0 + 1, k, 0:1, 0:2])
                nc.vector.tensor_copy(tge[p0 : p0 + 1, k, 0:1, 48:50], tge[p0 : p0 + 1, k, 0:1, 48:50])
                nc.vector.tensor_copy(tgo[p0 : p0 + 1, k, 0:1, 49:51], tgo[p0 : p0 + 1, k, 0:1, 49:51])
            nc.vector.tensor_copy(tge[0:1, 0, 0:1, 0:2], tge[0:1, 0, 0:1, 0:2])
            nc.vector.tensor_copy(tgo[0:1, 0, 0:1, 0:2], tgo[0:1, 0, 0:1, 0:2])

            for t in range(NT):
                prods = []
                for j in range(3):
                    d = 3 * t + j
                    tp, off = (tgo, 49 - d) if d & 1 else (tge, 48 - d)
                    prod = prodp.tile([128, NCH, HB, W], bf16, tag="prod")
                    nc.vector.tensor_mul(
                        prod[:], rf[:], tp[:, :, :, off : off + W]
                    )
                    prods.append(prod)

                for r in range(NR):
                    ps = psump.tile([120, 4, W], f32, tag="ps")
                    if r < 4:
                        h0 = 4 * r
                        mms = [(0, 0, 0, 128), (1, 1, 0, 128), (2, 2, 0, 64)]
                    else:
                        h0 = 4 * r - 16
                        mms = [(2, 3, 64, 64), (3, 4, 0, 128), (4, 5, 0, 128)]
                    # The MM instruction has a single sync-wait slot. The
                    # first matmul of the triple's first region would need
                    # two (psum WAR vs ScalarE evac + DVE prod-ready), so a
                    # K=1 zero matmul takes the WAR; later matmuls then ride
                    # PE program order.
                    if r == 0:
                        nc.tensor.matmul(
                            ps[:, :, :], zer_lhs[:, :], zer_rhs[:, :, :],
                            start=True, stop=False,
                        )
                    for j in range(3):
                        for i, (k, col, p0, np_) in enumerate(mms):
                            nc.tensor.matmul(
                                ps[:, :, :],
                                ones_sb[p0 : p0 + np_, col, j, :],
                                prods[j][p0 : p0 + np_, k, h0 : h0 + 4, :],
                                start=(r != 0 and j == 0 and i == 0),
                                stop=(j == 2 and i == 2),
                            )
                    ob = outp.tile([120, 4, W], f32, tag="ob")
                    nc.scalar.copy(ob[:], ps[:])
                    nc.sync.dma_start(
                        out_d[:, 3 * t : 3 * t + 3, 4 * r : 4 * r + 4, :].transpose(
                            [1, 0, 2, 3]
                        ),
                        ob[:, :, :],
                    )
    return nc


def _split_multi_waits(nc):
    """Legalize for this walrus: each TPB instruction struct has ONE sync-wait
    slot ("Too many sync wait commands" otherwise). Hoist all but the last
    wait of any multi-wait instruction onto standalone EventSemaphore
    instructions on the same engine queue, inserted just before it."""
    import concourse.mybir as mybir

    n = 0
    for fn in nc.m.functions:
        for blk in fn.blocks:
            insts = blk.instructions
            i = 0
            while i < len(insts):
                inst = insts[i]
                si = getattr(inst, "sync_info", None)
                if si is not None and len(si.on_wait) > 1:
                    waits = list(si.on_wait)
                    for w in waits[:-1]:
                        ev = mybir.InstEventSemaphore()
                        ev.engine = inst.engine
                        ev.name = f"I-evw{n}"
                        n += 1
                        ev.sync_info = mybir.SyncInfo(on_wait=[w], on_update=[])
                        insts.insert(i, ev)
                        i += 1
                    inst.sync_info = mybir.SyncInfo(
                        on_wait=[waits[-1]], on_update=list(si.on_update)
                    )
                i += 1
    return nc


def _get_built():
    if "nc" not in _CACHE:
        _CACHE["nc"] = _split_multi_waits(_build_nc())
        _CACHE["ones"] = _make_ones()
    return _CACHE["nc"], _CACHE["ones"]


def _kernel_numpy(ref, tgt, maxdisp, num_group):
    """Host fallback — guaranteed-correct grouped correlation volume."""
    cg = C // num_group
    r = ref.reshape(B, num_group, cg, H, W)
    out = np.zeros((B, num_group, maxdisp, H, W), np.float32)
    for d in range(maxdisp):
        t = np.zeros_like(tgt)
        if d:
            t[..., d:] = tgt[..., : W - d]
        else:
            t[...] = tgt
        tg = t.reshape(B, num_group, cg, H, W)
        out[:, :, d] = (r * tg).mean(axis=2)
    return out


def _kernel_device(ref, tgt):
    global LAST_RESULT
    from concourse import bass_utils

    nc, ones = _get_built()
    in_maps = []
    for i in range(NCORES):
        b, hh = divmod(i, 2)
        h0 = hh * Hc
        in_maps.append(
            {
                "ref": np.ascontiguousarray(ref[b, :, h0 : h0 + Hc, :]),
                "tgt": np.ascontiguousarray(tgt[b, :, h0 : h0 + Hc, :]),
                "ones": ones,
            }
        )

    trace = bool(int(os.environ.get("KTRACE", "0")))
    res = bass_utils.run_bass_kernel_spmd(
        nc, in_maps, list(range(NCORES)), trace=trace
    )
    LAST_RESULT = res

    out = np.empty((B, G, D, H, W), dtype=np.float32)
    for i in range(NCORES):
        b, hh = divmod(i, 2)
        out[b, :, :, hh * Hc : (hh + 1) * Hc, :] = res.results[i]["out"]
    return out


def kernel(refimg_fea, targetimg_fea, maxdisp=48, num_group=40):
    ref = np.asarray(refimg_fea, dtype=np.float32)
    tgt = np.asarray(targetimg_fea, dtype=np.float32)
    assert ref.shape == (B, C, H, W) and tgt.shape == (B, C, H, W)
    assert int(maxdisp) == D and int(num_group) == G

    try:
        return _kernel_device(ref, tgt)
    except Exception as e:  # device/compile failure: never return garbage
        sys.stderr.write(f"kernel: device path failed ({e!r}); numpy fallback\n")
        return _kernel_numpy(ref, tgt, int(maxdisp), int(num_group))


# revision 11
# speedup vs baseline: 1.7628x; 1.7628x over previous
"""Trainium2 Bass kernel for grouped-correlation cost volume (GwcNet style).

cost[b,g,d,h,w] = mean_{c in group g}( ref[b,c,h,w] * tgt[b,c,h,w-d] ), 0 if w<d

Hardcoded problem size: B=4, C=320, H=64, W=128, D=48, G=40 (cg=8), f32.
Sharding: 8 cores = (4 batches) x (2 halves of H). Each core computes its
[G, D, 32, W] shard; host reassembles.

Per-core dataflow (engines balanced, DVE does ONLY the multiplies):
  - inputs are cast f32->bf16 during the load DMA (SWDGE) into a 5-chunk
    packing that keeps all 128 partitions busy: chunks = (ch0:128 r0:16),
    (ch128:256 r0:16), (ch256:320 r0:16 | ch0:64 r16:32), (ch64:192 r16:32),
    (ch192:320 r16:32). tgt is loaded twice into zero-padded tiles at even
    and odd column parity so any disparity shift is a 4B-aligned slice
    (DVE 2x mode).
  - per d: ONE DVE tensor_mul over the whole 5-chunk stack [128, 5, 16, 128].
  - group-sum on PE: per d, 5 full-K=128 matmuls with block-structured (1/8)
    ones into an M=80 PSUM layout: out partition m<40 -> group m rows 0:16,
    m>=40 -> group m-40 rows 16:32 (the mixed chunk 2 contributes both halves
    in a single matmul).
  - ScalarE (ACT) evacuates PSUM->SBUF; HWDGE DMAs stream SBUF->DRAM out.
"""

import os
import sys

if "/opt/trn_rl_repo" not in sys.path:
    sys.path.insert(0, "/opt/trn_rl_repo")

import numpy as np

B, C, H, W = 4, 320, 64, 128
D, G, CG = 48, 40, 8
NCORES = 8
Hc = H // 2      # 32 rows of h per core
NCH = 5          # channel-row chunks, each 128 partitions
HB = 16          # h rows per chunk
PADW_E = 176     # even-parity padded tgt width (data at cols 48..176)
PADW_O = 178     # odd-parity padded tgt width (data at cols 49..177)
RT = 4           # output rows (per 16-row half) covered by one PSUM tile

# (chunk k, part0, parts, ch0, row0) for the input loads
_LOADS = [
    (0, 0, 128, 0, 0),
    (1, 0, 128, 128, 0),
    (2, 0, 64, 256, 0),
    (2, 64, 64, 0, 16),
    (3, 0, 128, 64, 16),
    (4, 0, 128, 192, 16),
]

_CACHE = {}
LAST_RESULT = None  # BassKernelResults of the most recent run (for profiling)


def _make_ones():
    import ml_dtypes

    # ones[p, k, m]: stationary for chunk k. Output partition m: m<40 ->
    # group m at rows 0:16 ("A" half), m>=40 -> group m-40 at rows 16:32 ("B").
    ones = np.zeros((128, NCH, 80), dtype=ml_dtypes.bfloat16)
    for p in range(128):
        ones[p, 0, p // CG] = 0.125                     # ch p        -> gA
        ones[p, 1, 16 + p // CG] = 0.125                # ch 128+p    -> gA
        if p < 64:
            ones[p, 2, 32 + p // CG] = 0.125            # ch 256+p    -> gA
        else:
            ones[p, 2, 40 + (p - 64) // CG] = 0.125     # ch p-64     -> gB
        ones[p, 3, 40 + 8 + p // CG] = 0.125            # ch 64+p     -> gB
        ones[p, 4, 40 + 24 + p // CG] = 0.125           # ch 192+p    -> gB
    return ones


def _build_nc():
    import concourse.bass as bass
    import concourse.mybir as mybir
    from concourse import tile

    nc = bass.Bass()
    ref_d = nc.dram_tensor("ref", [C, Hc, W], mybir.dt.float32, kind="ExternalInput")
    tgt_d = nc.dram_tensor("tgt", [C, Hc, W], mybir.dt.float32, kind="ExternalInput")
    ones_d = nc.dram_tensor("ones", [128, NCH, 80], mybir.dt.bfloat16, kind="ExternalInput")
    out_d = nc.dram_tensor("out", [G, D, Hc, W], mybir.dt.float32, kind="ExternalOutput")

    bf16 = mybir.dt.bfloat16
    f32 = mybir.dt.float32
    ntile = Hc // 2 // RT  # psum tiles per d

    with tile.TileContext(nc) as tc:
        with (
            tc.tile_pool(name="const", bufs=1) as constp,
            tc.tile_pool(name="inp", bufs=1) as inp,
            tc.tile_pool(name="prodp", bufs=4) as prodp,
            tc.tile_pool(name="outp", bufs=3) as outp,
            tc.tile_pool(name="psum", bufs=2 * (4 // (RT // 4)), space="PSUM") as psump,
        ):
            ones_sb = constp.tile([128, NCH, 80], bf16)
            nc.sync.dma_start(ones_sb[:], ones_d[:])

            rf = inp.tile([128, NCH, HB, W], bf16)
            tge = inp.tile([128, NCH, HB, PADW_E], bf16)
            tgo = inp.tile([128, NCH, HB, PADW_O], bf16)

            # zero the padded tgt tiles (pads must be 0), then cast-load the
            # data columns over them. gpsimd so DVE stays free.
            nc.gpsimd.memset(tge[:], 0.0)
            nc.gpsimd.memset(tgo[:], 0.0)
            for k, p0, np_, c0, r0 in _LOADS:
                src = lambda t: t[c0 : c0 + np_, r0 : r0 + HB, :]
                nc.gpsimd.dma_start(rf[p0 : p0 + np_, k, :, :], src(ref_d))
                nc.gpsimd.dma_start(tge[p0 : p0 + np_, k, :, 48 : 48 + W], src(tgt_d))
                nc.gpsimd.dma_start(tgo[p0 : p0 + np_, k, :, 49 : 49 + W], src(tgt_d))

            # tiny DVE joins, one per input writer: later DVE instructions
            # then ride program order instead of collecting DMA waits.
            for k, p0, np_, c0, r0 in _LOADS:
                nc.vector.tensor_copy(rf[p0 : p0 + 1, k, 0:1, 0:2], rf[p0 : p0 + 1, k, 0:1, 0:2])
                nc.vector.tensor_copy(tge[p0 : p0 + 1, k, 0:1, 48:50], tge[p0 : p0 + 1, k, 0:1, 48:50])
                nc.vector.tensor_copy(tgo[p0 : p0 + 1, k, 0:1, 49:51], tgo[p0 : p0 + 1, k, 0:1, 49:51])
            nc.vector.tensor_copy(tge[0:1, 0, 0:1, 0:2], tge[0:1, 0, 0:1, 0:2])
            nc.vector.tensor_copy(tgo[0:1, 0, 0:1, 0:2], tgo[0:1, 0, 0:1, 0:2])

            for d in range(D):
                tp, off = (tgo, 49 - d) if d & 1 else (tge, 48 - d)
                prod = prodp.tile([128, NCH, HB, W], bf16, tag="prod")
                nc.vector.tensor_mul(prod[:], rf[:], tp[:, :, :, off : off + W])

                for ti in range(ntile):
                    h0 = ti * RT
                    ps = psump.tile([80, RT, W], f32, tag="ps")
                    for k in range(NCH):
                        nc.tensor.matmul(
                            ps[:, :, :],
                            ones_sb[:, k, :],
                            prod[:, k, h0 : h0 + RT, :],
                            start=(k == 0),
                            stop=(k == NCH - 1),
                        )
                    ob = outp.tile([80, RT, W], f32, tag="ob")
                    nc.scalar.copy(ob[:], ps[:])
                    nc.sync.dma_start(
                        out_d[:, d, h0 : h0 + RT, :], ob[0:40, :, :]
                    )
                    nc.sync.dma_start(
                        out_d[:, d, 16 + h0 : 16 + h0 + RT, :], ob[40:80, :, :]
                    )
    return nc


def _split_multi_waits(nc):
    """Legalize for this walrus: each TPB instruction struct has ONE sync-wait
    slot ("Too many sync wait commands" otherwise). Hoist all but the last
    wait of any multi-wait instruction onto standalone EventSemaphore
    instructions on the same engine queue, inserted just before it."""
    import concourse.mybir as mybir

    n = 0
    for fn in nc.m.functions:
        for blk in fn.blocks:
            insts = blk.instructions
            i = 0
            while i < len(insts):
                inst = insts[i]
                si = getattr(inst, "sync_info", None)
                if si is not None and len(si.on_wait) > 1:
                    waits = list(si.on_wait)
                    for w in waits[:-1]:
                        ev = mybir.InstEventSemaphore()
                        ev.engine = inst.engine
                        ev.name = f"I-evw{n}"
                        n += 1
                        ev.sync_info = mybir.SyncInfo(on_wait=[w], on_update=[])
                        insts.insert(i, ev)
                        i += 1
                    inst.sync_info = mybir.SyncInfo(
                        on_wait=[waits[-1]], on_update=list(si.on_update)
                    )
                i += 1
    return nc


def _get_built():
    if "nc" not in _CACHE:
        _CACHE["nc"] = _split_multi_waits(_build_nc())
        _CACHE["ones"] = _make_ones()
    return _CACHE["nc"], _CACHE["ones"]


def _kernel_numpy(ref, tgt, maxdisp, num_group):
    """Host fallback — guaranteed-correct grouped correlation volume."""
    cg = C // num_group
    r = ref.reshape(B, num_group, cg, H, W)
    out = np.zeros((B, num_group, maxdisp, H, W), np.float32)
    for d in range(maxdisp):
        t = np.zeros_like(tgt)
        if d:
            t[..., d:] = tgt[..., : W - d]
        else:
            t[...] = tgt
        tg = t.reshape(B, num_group, cg, H, W)
        out[:, :, d] = (r * tg).mean(axis=2)
    return out


def _kernel_device(ref, tgt):
    global LAST_RESULT
    from concourse import bass_utils

    nc, ones = _get_built()
    in_maps = []
    for i in range(NCORES):
        b, hh = divmod(i, 2)
        h0 = hh * Hc
        in_maps.append(
            {
                "ref": np.ascontiguousarray(ref[b, :, h0 : h0 + Hc, :]),
                "tgt": np.ascontiguousarray(tgt[b, :, h0 : h0 + Hc, :]),
                "ones": ones,
            }
        )

    trace = bool(int(os.environ.get("KTRACE", "0")))
    res = bass_utils.run_bass_kernel_spmd(
        nc, in_maps, list(range(NCORES)), trace=trace
    )
    LAST_RESULT = res

    out = np.empty((B, G, D, H, W), dtype=np.float32)
    for i in range(NCORES):
        b, hh = divmod(i, 2)
        out[b, :, :, hh * Hc : (hh + 1) * Hc, :] = res.results[i]["out"]
    return out


def kernel(refimg_fea, targetimg_fea, maxdisp=48, num_group=40):
    ref = np.asarray(refimg_fea, dtype=np.float32)
    tgt = np.asarray(targetimg_fea, dtype=np.float32)
    assert ref.shape == (B, C, H, W) and tgt.shape == (B, C, H, W)
    assert int(maxdisp) == D and int(num_group) == G

    try:
        return _kernel_device(ref, tgt)
    except Exception as e:  # device/compile failure: never return garbage
        sys.stderr.write(f"kernel: device path failed ({e!r}); numpy fallback\n")
        return _kernel_numpy(ref, tgt, int(maxdisp), int(num_group))
